# revision 6
# baseline (speedup 1.0000x reference)
"""Trainium2 Bass kernel for nn_KlindtReadoutPerChannel2D.

Reference computation:
    out[b, n] = sum_{c,p} x[b,c,p] * mask_weights[p,c,n] * readout_weights[c,n]
with B=256, C=64, H=W=36 (P=1296), N=2000.

Strategy:
  * Fold readout_weights into mask_weights on-device (cheap DVE scaling),
    turning the whole op into one large matmul
        out[b, n] = sum_k xt[k, b] * (mask[k, n] * scale[k, n])
    over the flattened contraction axis k = (p, c).
  * Shard the CONTRACTION (channel) axis across the 8 NeuronCores:
    each core takes 8 of the 64 channels, computes a partial [256, 2000]
    output, and the host sums the 8 partials.  This minimizes HBM traffic:
    every core reads only its own slice of both x and mask_weights
    (~93 MB/core, the aggregate-minimum), unlike batch- or neuron-axis
    sharding which would replicate mask or x reads.
  * k is laid out p-major (k = p*8 + c_local) so that within every
    128-row k-tile the channel of row r is (r % 8).  A single [128, 2000]
    broadcast tile of readout rows then scales *every* mask k-tile.
  * Matmuls run as float32r (full PE rate for free-dim >= 256, vs 4x
    slower for plain fp32), accumulating in fp32 PSUM across all 81
    k-tiles; all 8 PSUM banks hold the [256, 2000] partial result.
"""

import numpy as np

B = 256
C = 64
P = 1296  # 36*36
N = 2000
NCORES = 8
CPC = C // NCORES  # channels per core = 8
KTOT = P * CPC  # per-core contraction length = 10368
KT = KTOT // 128  # 81 k-tiles
NB = 500  # matmul free-dim (PSUM bank holds 512 fp32)
NJ = N // NB  # 4 n-blocks
MT = B // 128  # 2 m-tiles

_PROGRAM = None


def _build_program():
    from contextlib import ExitStack

    from concourse import bacc, mybir, tile

    nc = bacc.Bacc("TRN2", target_bir_lowering=False, debug=False)
    f32 = mybir.dt.float32
    f32r = mybir.dt.float32r

    # xt feeds the matmul directly as the FP32r stationary operand, so it is
    # declared float32r end-to-end (same bits as fp32 in DRAM/SBUF).
    xt_d = nc.dram_tensor("xt", (KTOT, B), f32r, kind="ExternalInput").ap()
    mask_d = nc.dram_tensor("mask", (KTOT, N), f32, kind="ExternalInput").ap()
    scale_d = nc.dram_tensor("scale", (128, N), f32, kind="ExternalInput").ap()
    out_d = nc.dram_tensor("out", (B, N), f32, kind="ExternalOutput").ap()

    with tile.TileContext(nc) as tc:
        with ExitStack() as ctx:
            const_pool = ctx.enter_context(tc.tile_pool(name="const", bufs=1))
            mask_pool = ctx.enter_context(tc.tile_pool(name="mask", bufs=4))
            w_pool = ctx.enter_context(tc.tile_pool(name="w", bufs=3))
            xt_pool = ctx.enter_context(tc.tile_pool(name="xt", bufs=4))
            out_pool = ctx.enter_context(tc.tile_pool(name="out", bufs=2))
            psum_pool = ctx.enter_context(
                tc.tile_pool(name="psum", bufs=1, space="PSUM")
            )

            scale_t = const_pool.tile([128, N], f32)
            nc.sync.dma_start(scale_t[:], scale_d[:])

            # One PSUM tile spanning all 8 banks: bank (m*NJ + j) holds
            # out[m*128:(m+1)*128, j*500:(j+1)*500] (512-aligned slots).
            acc = psum_pool.tile([128, 8 * 512], f32)

            for k in range(KT):
                mask_t = mask_pool.tile([128, N], f32)
                nc.sync.dma_start(mask_t[:], mask_d[k * 128 : (k + 1) * 128, :])
                xt_t = xt_pool.tile([128, B], f32r)
                nc.sync.dma_start(xt_t[:], xt_d[k * 128 : (k + 1) * 128, :])

                # Output dtype f32r: the DVE rounds to FP32r precision, as
                # required for operands consumed by FP32r matmuls.
                w_t = w_pool.tile([128, N], f32r)
                nc.vector.tensor_mul(w_t[:], mask_t[:], scale_t[:])

                for m in range(MT):
                    lhsT = xt_t[:, m * 128 : (m + 1) * 128]
                    for j in range(NJ):
                        nc.tensor.matmul(
                            acc[:, (m * NJ + j) * 512 : (m * NJ + j) * 512 + NB],
                            lhsT,
                            w_t[:, j * NB : (j + 1) * NB],
                            start=(k == 0),
                            stop=(k == KT - 1),
                        )

            for m in range(MT):
                for j in range(NJ):
                    o_t = out_pool.tile([128, NB], f32)
                    nc.vector.tensor_copy(
                        o_t[:], acc[:, (m * NJ + j) * 512 : (m * NJ + j) * 512 + NB]
                    )
                    nc.sync.dma_start(
                        out_d[m * 128 : (m + 1) * 128, j * NB : (j + 1) * NB], o_t[:]
                    )

    nc.compile()
    return nc


def _make_in_maps(x, mask_weights, readout_weights):
    x_flat = np.asarray(x, dtype=np.float32).reshape(B, C, P)
    mask_weights = np.asarray(mask_weights, dtype=np.float32)
    readout_weights = np.asarray(readout_weights, dtype=np.float32)
    in_maps = []
    for core in range(NCORES):
        cs = slice(core * CPC, (core + 1) * CPC)
        # xt[k, b] with k = p*CPC + c_local  (p-major)
        xt = np.ascontiguousarray(
            x_flat[:, cs, :].transpose(2, 1, 0).reshape(KTOT, B)
        )
        # mask is already (P, C, N): slicing channels keeps p-major k order
        mask_s = np.ascontiguousarray(mask_weights[:, cs, :]).reshape(KTOT, N)
        # scale row r = readout[core's channel (r % CPC)]
        scale = np.ascontiguousarray(
            np.tile(readout_weights[cs, :], (128 // CPC, 1))
        )
        in_maps.append({"xt": xt, "mask": mask_s, "scale": scale})
    return in_maps


def _get_program():
    global _PROGRAM
    if _PROGRAM is None:
        _PROGRAM = _build_program()
    return _PROGRAM


def run_sharded(in_maps, **kwargs):
    from concourse.bass_utils import run_bass_kernel_spmd

    nc = _get_program()
    return run_bass_kernel_spmd(nc, in_maps, core_ids=list(range(NCORES)), **kwargs)


def kernel(x, mask_weights, readout_weights):
    in_maps = _make_in_maps(x, mask_weights, readout_weights)
    res = run_sharded(in_maps)
    out = np.zeros((B, N), dtype=np.float64)
    for r in res.results:
        out += r["out"]
    return out.astype(np.float32)


# revision 10
# speedup vs baseline: 1.4166x; 1.4166x over previous
"""Trainium2 Bass kernel for nn_KlindtReadoutPerChannel2D.

Reference computation:
    out[b, n] = sum_{c,p} x[b,c,p] * mask_weights[p,c,n] * readout_weights[c,n]
with B=256, C=64, H=W=36 (P=1296), N=2000.

Strategy:
  * Fold readout_weights into mask_weights on-device (cheap DVE scaling),
    turning the whole op into one large matmul
        out[b, n] = sum_k xt[k, b] * (mask[k, n] * scale[k, n])
    over the flattened contraction axis k = (p, c).
  * Shard the CONTRACTION (channel) axis across the 8 NeuronCores:
    each core takes 8 of the 64 channels, computes a partial [256, 2000]
    output, and the host sums the 8 partials.  This minimizes HBM traffic:
    every core reads only its own slice of both x and mask_weights
    (~93 MB/core, the aggregate-minimum), unlike batch- or neuron-axis
    sharding which would replicate mask or x reads.
  * k is laid out p-major (k = p*8 + c_local) so that within every
    128-row k-tile the channel of row r is (r % 8).  A single [128, 2000]
    broadcast tile of readout rows then scales *every* mask k-tile.
  * Matmuls run as float32r (full PE rate for free-dim >= 256, vs 4x
    slower for plain fp32), accumulating in fp32 PSUM across all 81
    k-tiles; all 8 PSUM banks hold the [256, 2000] partial result.
"""

import numpy as np

B = 256
C = 64
P = 1296  # 36*36
N = 2000
NCORES = 8
CPC = C // NCORES  # channels per core = 8
KTOT = P * CPC  # per-core contraction length = 10368
KT = KTOT // 128  # 81 k-tiles
NB = 500  # matmul free-dim (PSUM bank holds 512 fp32)
NJ = N // NB  # 4 n-blocks
MT = B // 128  # 2 m-tiles

_PROGRAM = {}


def _build_program(repeats=1):
    from contextlib import ExitStack

    from concourse import bacc, mybir, tile

    nc = bacc.Bacc("TRN2", target_bir_lowering=False, debug=False)
    f32 = mybir.dt.float32
    f32r = mybir.dt.float32r

    # xt feeds the matmul directly as the FP32r stationary operand, so it is
    # declared float32r end-to-end (same bits as fp32 in DRAM/SBUF).
    xt_d = nc.dram_tensor("xt", (KTOT, B), f32r, kind="ExternalInput").ap()
    mask_d = nc.dram_tensor("mask", (KTOT, N), f32, kind="ExternalInput").ap()
    scale_d = nc.dram_tensor("scale", (128, N), f32, kind="ExternalInput").ap()
    out_d = nc.dram_tensor("out", (B, N), f32, kind="ExternalOutput").ap()

    with tile.TileContext(nc) as tc:
        with ExitStack() as ctx:
            const_pool = ctx.enter_context(tc.tile_pool(name="const", bufs=1))
            mask_pool = ctx.enter_context(tc.tile_pool(name="mask", bufs=4))
            w_pool = ctx.enter_context(tc.tile_pool(name="w", bufs=3))
            xt_pool = ctx.enter_context(tc.tile_pool(name="xt", bufs=4))
            out_pool = ctx.enter_context(tc.tile_pool(name="out", bufs=2))
            psum_pool = ctx.enter_context(
                tc.tile_pool(name="psum", bufs=1, space="PSUM")
            )

            scale_t = const_pool.tile([128, N], f32)
            nc.sync.dma_start(scale_t[:], scale_d[:])

            # One PSUM tile spanning all 8 banks: bank (m*NJ + j) holds
            # out[m*128:(m+1)*128, j*500:(j+1)*500] (512-aligned slots).
            acc = psum_pool.tile([128, 8 * 512], f32)

            for _rep in range(repeats):
                _loop_body(nc, tc, mask_pool, w_pool, xt_pool, out_pool,
                           mask_d, xt_d, out_d, scale_t, acc, f32, f32r)

    nc.compile()
    return nc


def _loop_body(nc, tc, mask_pool, w_pool, xt_pool, out_pool,
               mask_d, xt_d, out_d, scale_t, acc, f32, f32r):
    if True:
            for k in range(KT):
                mask_t = mask_pool.tile([128, N], f32)
                nc.sync.dma_start(mask_t[:], mask_d[k * 128 : (k + 1) * 128, :])
                xt_t = xt_pool.tile([128, B], f32r)
                nc.sync.dma_start(xt_t[:], xt_d[k * 128 : (k + 1) * 128, :])

                # Output dtype f32r: the DVE rounds to FP32r precision, as
                # required for operands consumed by FP32r matmuls.
                w_t = w_pool.tile([128, N], f32r)
                nc.vector.tensor_mul(w_t[:], mask_t[:], scale_t[:])

                for m in range(MT):
                    lhsT = xt_t[:, m * 128 : (m + 1) * 128]
                    for j in range(NJ):
                        nc.tensor.matmul(
                            acc[:, (m * NJ + j) * 512 : (m * NJ + j) * 512 + NB],
                            lhsT,
                            w_t[:, j * NB : (j + 1) * NB],
                            start=(k == 0),
                            stop=(k == KT - 1),
                        )

            for m in range(MT):
                for j in range(NJ):
                    o_t = out_pool.tile([128, NB], f32)
                    nc.vector.tensor_copy(
                        o_t[:], acc[:, (m * NJ + j) * 512 : (m * NJ + j) * 512 + NB]
                    )
                    nc.sync.dma_start(
                        out_d[m * 128 : (m + 1) * 128, j * NB : (j + 1) * NB], o_t[:]
                    )


def _make_in_maps(x, mask_weights, readout_weights):
    x_flat = np.asarray(x, dtype=np.float32).reshape(B, C, P)
    mask_weights = np.asarray(mask_weights, dtype=np.float32)
    readout_weights = np.asarray(readout_weights, dtype=np.float32)
    in_maps = []
    for core in range(NCORES):
        cs = slice(core * CPC, (core + 1) * CPC)
        # xt[k, b] with k = p*CPC + c_local  (p-major)
        xt = np.ascontiguousarray(
            x_flat[:, cs, :].transpose(2, 1, 0).reshape(KTOT, B)
        )
        # mask is already (P, C, N): slicing channels keeps p-major k order
        mask_s = np.ascontiguousarray(mask_weights[:, cs, :]).reshape(KTOT, N)
        # scale row r = readout[core's channel (r % CPC)]
        scale = np.ascontiguousarray(
            np.tile(readout_weights[cs, :], (128 // CPC, 1))
        )
        in_maps.append({"xt": xt, "mask": mask_s, "scale": scale})
    return in_maps


def _get_program(repeats=1):
    if repeats not in _PROGRAM:
        _PROGRAM[repeats] = _build_program(repeats)
    return _PROGRAM[repeats]


def run_sharded(in_maps, **kwargs):
    from concourse.bass_utils import run_bass_kernel_spmd

    nc = _get_program()
    return run_bass_kernel_spmd(nc, in_maps, core_ids=list(range(NCORES)), **kwargs)


def kernel(x, mask_weights, readout_weights):
    in_maps = _make_in_maps(x, mask_weights, readout_weights)
    res = run_sharded(in_maps)
    out = np.zeros((B, N), dtype=np.float64)
    for r in res.results:
        out += r["out"]
    return out.astype(np.float32)


# revision 11
# speedup vs baseline: 24.6598x; 17.4076x over previous
"""Trainium2 Bass kernel for nn_KlindtReadoutPerChannel2D.

Reference computation:
    out[b, n] = sum_{c,p} x[b,c,p] * mask_weights[p,c,n] * readout_weights[c,n]
with B=256, C=64, H=W=36 (P=1296), N=2000.

Strategy:
  * Fold readout_weights into mask_weights on-device (cheap DVE scaling),
    turning the whole op into one large matmul
        out[b, n] = sum_k xt[k, b] * (mask[k, n] * scale[k, n])
    over the flattened contraction axis k = (p, c).
  * Shard the CONTRACTION (channel) axis across the 8 NeuronCores:
    each core takes 8 of the 64 channels, computes a partial [256, 2000]
    output, and the host sums the 8 partials.  This minimizes HBM traffic:
    every core reads only its own slice of both x and mask_weights,
    unlike batch- or neuron-axis sharding which would replicate mask or x
    reads 8x.
  * k is laid out p-major (k = p*8 + c_local) so that within every
    128-row k-tile the channel of row r is (r % 8).  A single [128, 2000]
    broadcast tile of readout rows then scales *every* mask k-tile.
  * x and mask are shipped as fp16 (halves the dominant HBM traffic; PE
    runs fp16 at full rate; PSUM accumulates in fp32).  The readout scale
    is pre-multiplied by 2**10 on the host so the scaled weights
    (~1e-4 in magnitude otherwise) stay in fp16 normal range; the final
    host-side gather divides the partial sums by 2**10.
  * The [256, 2000] fp32 partial output lives across all 8 PSUM banks and
    accumulates over all 81 contraction k-tiles without evacuation.
"""

import numpy as np

B = 256
C = 64
P = 1296  # 36*36
N = 2000
NCORES = 8
CPC = C // NCORES  # channels per core = 8
KTOT = P * CPC  # per-core contraction length = 10368
KT = KTOT // 128  # 81 k-tiles
NB = 500  # matmul free-dim (PSUM bank holds 512 fp32)
NJ = N // NB  # 4 n-blocks
MT = B // 128  # 2 m-tiles
SCALE_SHIFT = 10  # host folds 2**10 into readout scale, removed after gather

_PROGRAM = {}


def _build_program(repeats=1):
    from contextlib import ExitStack

    from concourse import bacc, mybir, tile

    nc = bacc.Bacc("TRN2", target_bir_lowering=False, debug=False)
    f32 = mybir.dt.float32
    f16 = mybir.dt.float16

    xt_d = nc.dram_tensor("xt", (KTOT, B), f16, kind="ExternalInput").ap()
    mask_d = nc.dram_tensor("mask", (KTOT, N), f16, kind="ExternalInput").ap()
    scale_d = nc.dram_tensor("scale", (128, N), f32, kind="ExternalInput").ap()
    out_d = nc.dram_tensor("out", (B, N), f32, kind="ExternalOutput").ap()

    with tile.TileContext(nc) as tc:
        with ExitStack() as ctx:
            const_pool = ctx.enter_context(tc.tile_pool(name="const", bufs=1))
            mask_pool = ctx.enter_context(tc.tile_pool(name="mask", bufs=4))
            w_pool = ctx.enter_context(tc.tile_pool(name="w", bufs=3))
            xt_pool = ctx.enter_context(tc.tile_pool(name="xt", bufs=4))
            out_pool = ctx.enter_context(tc.tile_pool(name="out", bufs=2))
            psum_pool = ctx.enter_context(
                tc.tile_pool(name="psum", bufs=1, space="PSUM")
            )

            scale_t = const_pool.tile([128, N], f32)
            nc.sync.dma_start(scale_t[:], scale_d[:])

            # One PSUM tile spanning all 8 banks: bank (m*NJ + j) holds
            # out[m*128:(m+1)*128, j*500:(j+1)*500] (512-aligned slots).
            acc = psum_pool.tile([128, 8 * 512], f32)

            for _rep in range(repeats):
                for k in range(KT):
                    mask_t = mask_pool.tile([128, N], f16)
                    nc.sync.dma_start(mask_t[:], mask_d[k * 128 : (k + 1) * 128, :])
                    xt_t = xt_pool.tile([128, B], f16)
                    nc.sync.dma_start(xt_t[:], xt_d[k * 128 : (k + 1) * 128, :])

                    w_t = w_pool.tile([128, N], f16)
                    nc.vector.tensor_mul(w_t[:], mask_t[:], scale_t[:])

                    for m in range(MT):
                        lhsT = xt_t[:, m * 128 : (m + 1) * 128]
                        for j in range(NJ):
                            nc.tensor.matmul(
                                acc[:, (m * NJ + j) * 512 : (m * NJ + j) * 512 + NB],
                                lhsT,
                                w_t[:, j * NB : (j + 1) * NB],
                                start=(k == 0),
                                stop=(k == KT - 1),
                            )

                for m in range(MT):
                    for j in range(NJ):
                        o_t = out_pool.tile([128, NB], f32)
                        nc.vector.tensor_copy(
                            o_t[:], acc[:, (m * NJ + j) * 512 : (m * NJ + j) * 512 + NB]
                        )
                        nc.sync.dma_start(
                            out_d[m * 128 : (m + 1) * 128, j * NB : (j + 1) * NB],
                            o_t[:],
                        )

    nc.compile()
    return nc


def _make_in_maps(x, mask_weights, readout_weights):
    x_flat = np.asarray(x, dtype=np.float32).reshape(B, C, P)
    mask_weights = np.asarray(mask_weights, dtype=np.float32)
    readout_weights = np.asarray(readout_weights, dtype=np.float32)
    in_maps = []
    for core in range(NCORES):
        cs = slice(core * CPC, (core + 1) * CPC)
        # xt[k, b] with k = p*CPC + c_local  (p-major)
        xt = np.ascontiguousarray(
            x_flat[:, cs, :].transpose(2, 1, 0).reshape(KTOT, B).astype(np.float16)
        )
        # mask is already (P, C, N): slicing channels keeps p-major k order
        mask_s = mask_weights[:, cs, :].astype(np.float16).reshape(KTOT, N)
        # scale row r = readout[core's channel (r % CPC)], pre-shifted so the
        # fp16 scaled weights stay in normal range
        scale = np.ascontiguousarray(
            np.tile(readout_weights[cs, :] * np.float32(2.0**SCALE_SHIFT),
                    (128 // CPC, 1))
        )
        in_maps.append({"xt": xt, "mask": mask_s, "scale": scale})
    return in_maps


def _get_program(repeats=1):
    if repeats not in _PROGRAM:
        _PROGRAM[repeats] = _build_program(repeats)
    return _PROGRAM[repeats]


def run_sharded(in_maps, **kwargs):
    from concourse.bass_utils import run_bass_kernel_spmd

    nc = _get_program()
    return run_bass_kernel_spmd(nc, in_maps, core_ids=list(range(NCORES)), **kwargs)


def kernel(x, mask_weights, readout_weights):
    in_maps = _make_in_maps(x, mask_weights, readout_weights)
    res = run_sharded(in_maps)
    out = np.zeros((B, N), dtype=np.float64)
    for r in res.results:
        out += r["out"]
    out *= 1.0 / 2.0**SCALE_SHIFT
    return out.astype(np.float32)
